# revision 1
# baseline (speedup 1.0000x reference)
"""Trainium2 Bass kernel for the CgpHmmCell forward log-likelihood.

Computes loglik[b] = log-likelihood of each observation sequence under an
HMM with A = softmax(A_kernel, axis=1), Bm = softmax(B_kernel, axis=0),
initial mass on state 0 — matching the stabilized log-domain reference scan.

Strategy
--------
Data-parallel over batch: core c owns sequences 4c..4c+3.  Within a core the
T=4096 scan is broken into 64 time-chunks of L=64 steps; each (seq, chunk)
pair is an independent "lane" (256 lanes/core) run in lockstep in the LINEAR
domain (f <- (f @ A) * e_t, with emissions prescaled so the log-magnitude
drift stays centered).  HMM forward recursions are exponentially forgetting
(direction error decays ~10x/step for this operator family), so each chunk
burns in for H=16 steps from a uniform init before its segment; per-chunk
log-growth u_c = lnSum(end) - lnSum(burn-in end) is exact after burn-in and
sums to the sequence log-likelihood.  Chunk 0 is re-seeded exactly (delta on
state 0) at the step its segment starts.

Lane state is kept transposed ([state, lane]): the per-step transition is 16
PE matmuls with A-tiles stationary (bf16), i-outer so each s_out accumulation
group owns a full PSUM bank and next-step matmuls overlap this step's
emission multiplies.  Emission rows are generated on-device from the one-hot
inputs: the fp32 one-hot is bulk-DMA-transposed as uint16 (the high half of
fp32 1.0 is exactly bf16 1.0), giving a partition-interleaved transposed
one-hot that is contracted against an odd-row-interleaved Bm.  The full
one-hot input is thus streamed from HBM once, which is the memory-bound part
of the computation.
"""

import sys

sys.path.insert(0, "/opt/trn_rl_repo")

import numpy as np

import concourse.bass as bass
import concourse.tile as tile
from concourse import bacc, mybir
from concourse.bass import ts
from concourse.bass_utils import run_bass_kernel_spmd
from concourse.masks import make_identity

# problem shapes (hardcoded per contract)
B, T, S, E = 32, 4096, 512, 128
NCORES = 8
BPC = B // NCORES          # sequences per core = 4
L = 64                     # chunk length
H = 8                      # burn-in steps
NCHUNK = T // L            # 64 chunks per sequence
LANES = BPC * NCHUNK       # 256 lanes per core
STEPS = H + L              # 80 lockstep steps
W = 8                      # one-hot DMA block for the burn-in prologue
LNS = float(np.log(128.0) - 0.05)   # emission prescale (log)
SJ = S // 128              # 4 state tiles
FP32 = mybir.dt.float32
BF16 = mybir.dt.bfloat16
U16 = mybir.dt.uint16

ABLATE = set()  # {'no_e', 'no_mm', 'no_mult'} for cost bisection
NG = 1                     # lane groups interleaved on the PE (1 or 2)
NL = LANES // NG           # lanes per group
REPS = 1                   # repeat whole scan (timing instrumentation)
SEG_EXT = 64               # segment steps fed by the PE-transpose E path


def build_program():
    nc = bacc.Bacc("TRN2", debug=False, num_devices=NCORES)

    x = nc.dram_tensor("x", [BPC, T, E], FP32, kind="ExternalInput")
    ak = nc.dram_tensor("ak", [S, S], FP32, kind="ExternalInput")
    bk = nc.dram_tensor("bk", [E, S], FP32, kind="ExternalInput")
    out = nc.dram_tensor("out", [1, BPC], FP32, kind="ExternalOutput")

    with tile.TileContext(nc) as tc:
        with (
            tc.tile_pool(name="singles", bufs=1) as singles,
            tc.tile_pool(name="prep", bufs=2) as prep,
            tc.tile_pool(name="xpool", bufs=2) as xpool,
            tc.tile_pool(name="xtsb", bufs=2) as xtsb,
            tc.tile_pool(name="esb", bufs=3) as esb,
            tc.tile_pool(name="phip", bufs=2) as phip,
            tc.tile_pool(name="main_ps", bufs=2, space="PSUM") as main_psp,
            tc.tile_pool(name="e_ps", bufs=2, space="PSUM") as e_psp,
        ):
            # ---------------- one-hot bulk transpose (segment steps) -------
            # xt_seg[h] [128, (b, t)] u16: partition p = u16 column 128h+p of
            # the fp32 row; odd partitions are bf16 one-hot values.
            xu = x.ap().bitcast(U16)          # [BPC, T, 256]
            xt_seg = None
            if "no_e" not in ABLATE and H + SEG_EXT < STEPS:
                xt_seg = [
                    singles.tile([128, BPC * T], U16, name=f"xt_seg{h}")
                    for h in range(2)
                ]
                for h in range(2):
                    for b in range(BPC):
                        nc.sync.dma_start_transpose(
                            xt_seg[h][:, b * T : (b + 1) * T],
                            xu[b, :, 128 * h : 128 * h + 128],
                        )

            # ---------------- one-time prep ----------------
            ident = singles.tile([128, 128], FP32)
            make_identity(nc, ident)

            # Bm = softmax(bk, axis=0) * exp(LNS), emissions (e) on partitions
            b_sb = prep.tile([E, S], FP32, tag="b_stage")
            nc.sync.dma_start(out=b_sb, in_=bk.ap())
            expb = prep.tile([E, S], FP32, tag="expb")
            nc.scalar.activation(out=expb, in_=b_sb, func=mybir.ActivationFunctionType.Exp)
            ones_col = singles.tile([E, 1], FP32)
            nc.vector.memset(ones_col, 1.0)
            zb_ps = e_psp.tile([1, S], FP32, tag="e_ps")
            nc.tensor.matmul(zb_ps, lhsT=ones_col, rhs=expb, start=True, stop=True)
            recb = prep.tile([1, S], FP32, tag="recb")
            nc.vector.reciprocal(out=recb, in_=zb_ps)
            nc.vector.tensor_scalar_mul(out=recb, in0=recb, scalar1=float(np.exp(LNS)))
            ones_row = singles.tile([1, 128], FP32)
            nc.vector.memset(ones_row, 1.0)
            bc_ps = e_psp.tile([128, S], FP32, tag="e_ps")
            nc.tensor.matmul(bc_ps, lhsT=ones_row, rhs=recb, start=True, stop=True)
            Bm_bf = singles.tile([E, S], BF16)
            nc.vector.tensor_tensor(
                out=Bm_bf, in0=expb, in1=bc_ps, op=mybir.AluOpType.mult
            )

            # interleaved Bm for the u16-transposed one-hot path: odd
            # partition 2r+1 of half h holds Bm[64h + r]; even partitions
            # (low fp32 halves, always zero in the one-hot) hold zeros.
            Bm_int = singles.tile([128, 2, S], BF16)
            for h in range(2):
                # permutation matrix P[e, p] = 1 iff p == 2e + 1 - 128h
                ph = prep.tile([128, 128], BF16, tag="ph")
                nc.gpsimd.memset(ph, 0.0)
                nc.gpsimd.affine_select(
                    out=ph, in_=ph, compare_op=mybir.AluOpType.not_equal,
                    fill=1.0, base=1 - 128 * h, channel_multiplier=2,
                    pattern=[[-1, 128]],
                )
                int_ps = e_psp.tile([128, S], FP32, tag="e_ps")
                nc.tensor.matmul(int_ps, lhsT=ph, rhs=Bm_bf, start=True, stop=True)
                nc.any.tensor_copy(out=Bm_int[:, h, :], in_=int_ps)

            ones_s = singles.tile([128, 1], BF16)
            nc.vector.memset(ones_s, 1.0)
            lnS_start = singles.tile([1, LANES], FP32)
            lnS_end = singles.tile([1, LANES], FP32)

            # ---------------- burn-in (prologue) E path ----------------
            x_ap = x.ap()
            xg_cur = {}

            def prologue_load(k0):
                """Stage one-hot rows for prologue steps k0..k0+W-1.

                Lane (b, c) at step k consumes t = c*L + (k - H); for k < H
                c>=1 lanes borrow from chunk c-1 and c=0 lanes clamp to t=0
                (garbage; re-seeded exactly at k == H)."""
                for g in range(2):
                    xg = xpool.tile([128, W, E], FP32, tag=f"x{g}")
                    for bb in range(2):
                        b = 2 * g + bb
                        xr = x_ap[b].rearrange("(c l) e -> c l e", l=L)
                        if k0 >= H:
                            nc.sync.dma_start(
                                out=xg[64 * bb : 64 * bb + 64],
                                in_=xr[:, k0 - H : k0 - H + W, :],
                            )
                            continue
                        nc.sync.dma_start(
                            out=xg[64 * bb + 1 : 64 * bb + 64],
                            in_=xr[0 : NCHUNK - 1, L + k0 - H : L + k0 - H + W, :],
                        )
                        src = x_ap[b, 0:1, :]
                        bcast = bass.AP(
                            tensor=src.tensor,
                            offset=src.offset,
                            ap=[[0, 1], [0, W], [1, E]],
                        )
                        nc.sync.dma_start(out=xg[64 * bb : 64 * bb + 1, :, :], in_=bcast)
                    xg_cur[g] = xg

            def e_transpose(k):
                """PE-transpose prologue one-hot for step k -> bf16 SBUF."""
                if k % W == 0:
                    prologue_load(k)
                xt_ps = e_psp.tile([128, 2, 128], FP32, tag="e_ps")
                xt_sb = xtsb.tile([128, 2 * 128], BF16, tag="xt_sb")
                for g in range(2):
                    nc.tensor.transpose(xt_ps[:, g, :], xg_cur[g][:, k % W, :], ident)
                for g in range(2):
                    nc.any.tensor_copy(out=xt_sb[:, ts(g, 128)], in_=xt_ps[:, g, :])
                return xt_sb

            BPG = BPC // NG   # sequences per lane group

            def e_matmul_half(k, xt_sb, lh, e_ps):
                """Emission rows for step k, lane group lh (lanes lh*NL..):
                e_ps[:, lh, j, :] = Bm[obs, s-tile j] (scaled)."""
                if k < H + SEG_EXT:
                    for j in range(SJ):
                        nc.tensor.matmul(
                            e_ps[:, lh, j, :],
                            lhsT=Bm_bf[:, ts(j, 128)],
                            rhs=xt_sb[:, lh * NL : lh * NL + NL],
                            start=True, stop=True,
                        )
                else:
                    w = k - H
                    for j in range(SJ):
                        for eh in range(2):
                            rhs = xt_seg[eh].bitcast(BF16).rearrange(
                                "p (b c w) -> p b c w", c=NCHUNK, w=L
                            )[:, BPG * lh : BPG * lh + BPG, :, w]
                            nc.tensor.matmul(
                                e_ps[:, lh, j, :],
                                lhsT=Bm_int[:, eh, ts(j, 128)],
                                rhs=rhs,
                                start=(eh == 0),
                                stop=(eh == 1),
                            )

            def e_matmul(k, xt_sb):
                e_ps = e_psp.tile([128, NG, SJ, NL], FP32, tag="e_ps")
                for lh in range(NG):
                    e_matmul_half(k, xt_sb, lh, e_ps)
                # walrus cannot encode two PSUM operands on one TensorTensor,
                # so emissions move to SBUF (bf16) on the otherwise-idle ACT.
                e_sb = esb.tile([128, NG, SJ, NL], BF16, tag="e_sb")
                nc.scalar.copy(out=e_sb, in_=e_ps)
                return e_sb

            # ---------------- main lockstep scan ----------------
            if "no_e" in ABLATE:
                e_const = singles.tile([128, 2, SJ, 128], FP32)
                nc.vector.memset(e_const, 0.01)
            for _rep in range(REPS):
              if "no_e" in ABLATE:
                e_q = xt_q = e_ps_q = None
              else:
                xt_q = [e_transpose(0), e_transpose(1)]
                e_ps_q = {}
                e_q = [e_matmul(0, xt_q[0])]

              if _rep == 0:
                # A-softmax emitted after E-warmup so it overlaps the
                # prologue DMA/transposes (A is first needed at step 1)
                # A = softmax(ak, axis=1), rows (s_in) on partitions -> bf16 tiles
                a_sb = prep.tile([128, SJ, S], FP32, tag="a_stage")
                nc.sync.dma_start(out=a_sb, in_=ak.ap().rearrange("(ko ki) m -> ki ko m", ki=128))
                A_bf = singles.tile([128, SJ, S], BF16)
                for ko in range(SJ):
                    expa = prep.tile([128, S], FP32, tag="expa")
                    zs = prep.tile([128, 1], FP32, tag="zs")
                    nc.scalar.activation(
                        out=expa, in_=a_sb[:, ko, :],
                        func=mybir.ActivationFunctionType.Exp, accum_out=zs,
                    )
                    rec = prep.tile([128, 1], FP32, tag="rec")
                    nc.vector.reciprocal(out=rec, in_=zs)
                    nc.vector.tensor_scalar_mul(out=A_bf[:, ko, :], in0=expa, scalar1=rec)

              phi = None
              for k in range(STEPS):
                def advance_e(lh):
                    if "no_e" in ABLATE:
                        return
                    if lh == 0:
                        if k + 2 < H + SEG_EXT:
                            xt_q.append(e_transpose(k + 2))
                        if k + 1 < STEPS:
                            e_ps_q[k + 1] = e_psp.tile(
                                [128, NG, SJ, NL], FP32, tag="e_ps",
                                name=f"e_ps_{k + 1}",
                            )
                    if k + 1 < STEPS:
                        e_matmul_half(
                            k + 1, xt_q[k + 1] if k + 1 < H + SEG_EXT else None,
                            lh, e_ps_q[k + 1],
                        )
                        if lh == NG - 1:
                            e_sb_n = esb.tile([128, NG, SJ, NL], BF16, tag="e_sb",
                                              name=f"e_sb_{k + 1}")
                            nc.scalar.copy(out=e_sb_n, in_=e_ps_q[k + 1])
                            e_q.append(e_sb_n)

                e_sb = e_const if "no_e" in ABLATE else e_q[k]

                # phi is split into two tiles (state chunks i=0,1 / i=2,3) so
                # Tile's per-tile dependency tracking lets next-step i-outer
                # matmul rounds start as soon as the matching multiply lands.
                phi_new = [
                    phip.tile([128, 2, LANES], BF16, tag=f"phi{jp}",
                              name=f"phi{jp}_{k}")
                    for jp in range(2)
                ]
                if k == 0:
                    for lh in range(NG):
                        advance_e(lh)
                    for jp in range(2):
                        for lh in range(NG):
                            nc.any.tensor_copy(
                                out=phi_new[jp][:, :, lh * NL : lh * NL + NL],
                                in_=e_sb[:, lh, 2 * jp : 2 * jp + 2, :],
                            )
                else:
                    # main PSUM is two double-buffered 1-bank tiles (one per
                    # j-pair): the jp0 multiply starts halfway through the
                    # matmul block and neither multiply blocks next step's
                    # matmuls (fresh buffers).
                    main_ps = [
                        main_psp.tile([128, NG, 2, NL], FP32, tag=f"main{jp}",
                                      name=f"main{jp}_{k}")
                        for jp in range(2)
                    ]
                    for lh in range(NG):
                        if "no_mm" not in ABLATE:
                            for j in range(SJ):
                                for i in range(SJ):
                                    nc.tensor.matmul(
                                        main_ps[j // 2][:, lh, j % 2, :],
                                        lhsT=A_bf[:, i, ts(j, 128)],
                                        rhs=phi[i // 2][:, i % 2,
                                                        lh * NL : lh * NL + NL],
                                        start=(i == 0),
                                        stop=(i == SJ - 1),
                                    )
                        advance_e(lh)
                        for jp in range(2):
                            if "no_mult" in ABLATE or "no_mm" in ABLATE:
                                nc.any.tensor_copy(
                                    out=phi_new[jp][:, :, lh * NL : lh * NL + NL],
                                    in_=e_sb[:, lh, 2 * jp : 2 * jp + 2, :],
                                )
                                continue
                            nc.any.tensor_tensor(
                                out=phi_new[jp][:, :, lh * NL : lh * NL + NL],
                                in0=main_ps[jp][:, lh, :, :],
                                in1=e_sb[:, lh, 2 * jp : 2 * jp + 2, :],
                                op=mybir.AluOpType.mult,
                            )

                if k == H:
                    # exact re-seed of chunk-0 lanes (cols 0,64,128,192):
                    # phi[:, lane] = delta(state 0) * e_t0[:, lane]
                    for jp in range(2):
                        for jo in range(2):
                            nc.vector.memset(
                                phi_new[jp][:, jo, :].rearrange(
                                    "p (b c) -> p b c", c=L
                                )[:, :, 0],
                                0.0,
                            )
                    for lh in range(NG):
                        nc.vector.tensor_copy(
                            out=phi_new[0][0:1, 0, :].rearrange(
                                "p (b c) -> p b c", c=L
                            )[:, BPG * lh : BPG * lh + BPG, 0],
                            in_=e_sb[0:1, lh, 0, :].rearrange(
                                "p (b c) -> p b c", c=L
                            )[:, :, 0],
                        )

                if k in (H - 1, STEPS - 1):
                    ck_ps = e_psp.tile([1, LANES], FP32, tag="e_ps")
                    for i in range(SJ):
                        nc.tensor.matmul(
                            ck_ps, lhsT=ones_s, rhs=phi_new[i // 2][:, i % 2, :],
                            start=(i == 0), stop=(i == SJ - 1),
                        )
                    tgt = lnS_start if k == H - 1 else lnS_end
                    nc.scalar.activation(
                        out=tgt, in_=ck_ps, func=mybir.ActivationFunctionType.Ln
                    )

                phi = phi_new

            # ---------------- combine ----------------
            u = prep.tile([1, LANES], FP32, tag="u")
            nc.vector.tensor_tensor(
                out=u, in0=lnS_end, in1=lnS_start, op=mybir.AluOpType.subtract
            )
            # chunk-0 lanes have no burn-in reference: u = lnS_end
            nc.vector.tensor_add(
                out=u.rearrange("p (b c) -> p b c", c=L)[:, :, 0],
                in0=u.rearrange("p (b c) -> p b c", c=L)[:, :, 0],
                in1=lnS_start.rearrange("p (b c) -> p b c", c=L)[:, :, 0],
            )
            ub = prep.tile([1, BPC], FP32, tag="ub")
            nc.vector.tensor_reduce(
                out=ub,
                in_=u.rearrange("p (b c) -> p b c", c=L),
                axis=mybir.AxisListType.X,
                op=mybir.AluOpType.add,
            )
            # undo the per-step prescale: every one of the T emissions was
            # multiplied by exp(LNS)
            nc.vector.tensor_scalar_add(out=ub, in0=ub, scalar1=float(-T * LNS))
            nc.sync.dma_start(out=out.ap(), in_=ub)

    nc.compile()
    return nc


_NC = None


def _get_nc():
    global _NC
    if _NC is None:
        _NC = build_program()
    return _NC


def kernel(inputs: np.ndarray, A_kernel: np.ndarray, B_kernel: np.ndarray) -> np.ndarray:
    nc = _get_nc()
    ak = np.ascontiguousarray(A_kernel, dtype=np.float32)
    bk = np.ascontiguousarray(B_kernel, dtype=np.float32)
    in_maps = [
        {
            "x": np.ascontiguousarray(inputs[BPC * c : BPC * (c + 1)], dtype=np.float32),
            "ak": ak,
            "bk": bk,
        }
        for c in range(NCORES)
    ]
    res = run_bass_kernel_spmd(nc, in_maps, core_ids=list(range(NCORES)))
    out = np.concatenate([res.results[c]["out"].reshape(BPC) for c in range(NCORES)])
    return out.reshape(B, 1).astype(np.float32)



# revision 7
# speedup vs baseline: 1.0608x; 1.0608x over previous
"""Trainium2 Bass kernel for the CgpHmmCell forward log-likelihood.

Computes loglik[b] = log-likelihood of each observation sequence under an
HMM with A = softmax(A_kernel, axis=1), Bm = softmax(B_kernel, axis=0),
initial mass on state 0 — matching the stabilized log-domain reference scan.

Strategy
--------
Data-parallel over batch: core c owns sequences 4c..4c+3.  Within a core the
T=4096 scan is broken into 64 time-chunks of L=64 steps; each (seq, chunk)
pair is an independent "lane" (256 lanes/core) run in lockstep in the LINEAR
domain (f <- (f @ A) * e_t).  HMM forward recursions are exponentially
forgetting, so each chunk burns in for H=8 steps from its predecessor's tail
before its segment; per-chunk log-growth u_c = lnSum(end) - lnSum(burn-in
end) is exact after burn-in and sums to the sequence log-likelihood.  Chunk 0
is re-seeded exactly (delta on state 0) at the step its segment starts.

The per-step transition runs entirely in fp8e4 (e4m3) on the PE with
DoubleRow perf mode: each DoubleRow matmul contracts two 128-deep k-tiles in
one instruction at 0.5 cycles/row, so the 512x512 A-apply for 256 lanes is 8
instructions (~1024 PE cycles) instead of 16 bf16 instructions (~4096).  A is
prescaled by 32 and emissions by exp(LNS)/32 so all fp8 operands sit in
e4m3's normal range; LNS = ln(128) gives the lane state a slight upward
per-step drift that keeps phi away from the e4m3 subnormal floor over the
72-step chunk life.  Emission rows are generated on-device from the one-hot
inputs: fp32 one-hot blocks are PE-transposed, cast to fp8, and contracted
against Bm (zero-padded DoubleRow slot).  Emissions stage through SBUF as
bf16 (PSUM cannot feed both operands of a TensorTensor); the phi-multiplies
are split per state-tile so the next step's A-matmuls unblock as early as
possible.
"""

import sys

sys.path.insert(0, "/opt/trn_rl_repo")

import numpy as np

import concourse.bass as bass
import concourse.tile as tile
from concourse import bacc, mybir
from concourse.bass import ts
from concourse.bass_utils import run_bass_kernel_spmd
from concourse.masks import make_identity

# problem shapes (hardcoded per contract)
B, T, S, E = 32, 4096, 512, 128
NCORES = 8
BPC = B // NCORES          # sequences per core = 4
L = 64                     # chunk length
H = 8                      # burn-in steps
NCHUNK = T // L            # 64 chunks per sequence
LANES = BPC * NCHUNK       # 256 lanes per core
STEPS = H + L              # 72 lockstep steps
W = 8                      # one-hot DMA block for the prologue loads
LNS = float(np.log(128.0))  # emission prescale (log); slight upward drift
SJ = S // 128              # 4 state tiles
ASCALE = 32.0              # fp8 prescale on A (counter-scaled into emissions)
CSEED = 256.0              # chunk-0 reseed boost (keeps fp8 phi in range)
FP32 = mybir.dt.float32
BF16 = mybir.dt.bfloat16
FP8 = mybir.dt.float8e4
FP8E5 = mybir.dt.float8e5
DR = mybir.MatmulPerfMode.DoubleRow


def _with_bcast2(ap, n=2):
    """Insert a stride-0 dim of size n after the partition dim."""
    return bass.AP(tensor=ap.tensor, offset=ap.offset,
                   ap=[ap.ap[0], [0, n]] + list(ap.ap[1:]))


def build_program():
    nc = bacc.Bacc("TRN2", debug=False, num_devices=NCORES)

    x = nc.dram_tensor("x", [BPC, T, E], FP32, kind="ExternalInput")
    ak = nc.dram_tensor("ak", [S, S], FP32, kind="ExternalInput")
    bk = nc.dram_tensor("bk", [E, S], FP32, kind="ExternalInput")
    out = nc.dram_tensor("out", [1, BPC], FP32, kind="ExternalOutput")

    with tile.TileContext(nc) as tc:
        with (
            tc.tile_pool(name="singles", bufs=1) as singles,
            tc.tile_pool(name="prep", bufs=2) as prep,
            tc.tile_pool(name="xpool", bufs=2) as xpool,
            tc.tile_pool(name="xtsb", bufs=3) as xtsb,
            tc.tile_pool(name="esb", bufs=3) as esb,
            tc.tile_pool(name="phip", bufs=2) as phip,
            tc.tile_pool(name="main_ps", bufs=1, space="PSUM") as main_psp,
            tc.tile_pool(name="e_ps", bufs=2, space="PSUM") as e_psp,
            tc.tile_pool(name="xt_ps", bufs=1, space="PSUM") as xt_psp,
        ):
            # ---------------- one-time prep ----------------
            ident = singles.tile([128, 128], FP32)
            make_identity(nc, ident)

            # Bm8[:, 0, j*128:] = softmax(bk, axis=0) * exp(LNS)/ASCALE; slot 1
            # zero (DoubleRow zero-pad).  Emissions e = Bm8.T @ onehot.
            b_sb = prep.tile([E, S], FP32, tag="b_stage")
            nc.sync.dma_start(out=b_sb, in_=bk.ap())
            expb = prep.tile([E, S], FP32, tag="expb")
            nc.scalar.activation(out=expb, in_=b_sb, func=mybir.ActivationFunctionType.Exp)
            ones_col = singles.tile([E, 1], FP32)
            nc.vector.memset(ones_col, 1.0)
            zb_t = e_psp.tile([128, SJ, LANES], FP32, tag="e_ps")
            zb = zb_t[:, :, :].rearrange("p j l -> p (j l)")
            nc.tensor.matmul(zb[0:1, 0:S], lhsT=ones_col, rhs=expb, start=True, stop=True)
            recb = prep.tile([1, S], FP32, tag="recb")
            nc.vector.reciprocal(out=recb, in_=zb[0:1, 0:S])
            nc.vector.tensor_scalar_mul(out=recb, in0=recb,
                                        scalar1=float(np.exp(LNS) / ASCALE))
            ones_row = singles.tile([1, 128], FP32)
            nc.vector.memset(ones_row, 1.0)
            bc_t = e_psp.tile([128, SJ, LANES], FP32, tag="e_ps")
            bc = bc_t[:, :, :].rearrange("p j l -> p (j l)")
            nc.tensor.matmul(bc[:, 0:S], lhsT=ones_row, rhs=recb, start=True, stop=True)
            Bm8 = singles.tile([E, 2, S], FP8)
            nc.gpsimd.memset(Bm8, 0.0)
            nc.vector.tensor_tensor(
                out=Bm8[:, 0, :], in0=expb, in1=bc[:, 0:S], op=mybir.AluOpType.mult
            )

            # A8[:, p, s, :] = softmax(ak, axis=1)[128*(2p+s) + ki, :] * ASCALE
            a_sb = prep.tile([128, SJ, S], FP32, tag="a_stage")
            nc.sync.dma_start(out=a_sb, in_=ak.ap().rearrange("(ko ki) m -> ki ko m", ki=128))
            A8 = singles.tile([128, 2, 2, S], FP8)
            for ko in range(SJ):
                expa = prep.tile([128, S], FP32, tag="expa")
                zs = prep.tile([128, 1], FP32, tag="zs")
                nc.scalar.activation(
                    out=expa, in_=a_sb[:, ko, :],
                    func=mybir.ActivationFunctionType.Exp, accum_out=zs,
                )
                rec = prep.tile([128, 1], FP32, tag="rec")
                nc.vector.reciprocal(out=rec, in_=zs)
                nc.vector.tensor_scalar_mul(out=rec, in0=rec, scalar1=float(ASCALE))
                nc.vector.tensor_scalar_mul(
                    out=A8[:, ko // 2, ko % 2, :], in0=expa, scalar1=rec
                )

            ones8 = singles.tile([128, 1], FP8)
            nc.vector.memset(ones8, 1.0)
            lnS_start = singles.tile([1, LANES], FP32)
            lnS_end = singles.tile([1, LANES], FP32)

            # ---------------- one-hot load + transpose pipeline ----------------
            x_ap = x.ap()
            xg_cur = {}

            def prologue_load(k0):
                """Stage one-hot rows for steps k0..k0+W-1.

                Lane (b, c) at step k consumes t = c*L + (k - H); for k < H
                c>=1 lanes borrow from chunk c-1 and c=0 lanes clamp to t=0
                (garbage; re-seeded exactly at k == H)."""
                for g in range(2):
                    xg = xpool.tile([128, W, E], FP32, tag=f"x{g}")
                    for bb in range(2):
                        b = 2 * g + bb
                        xr = x_ap[b].rearrange("(c l) e -> c l e", l=L)
                        if k0 >= H:
                            nc.sync.dma_start(
                                out=xg[64 * bb : 64 * bb + 64],
                                in_=xr[:, k0 - H : k0 - H + W, :],
                            )
                            continue
                        nc.sync.dma_start(
                            out=xg[64 * bb + 1 : 64 * bb + 64],
                            in_=xr[0 : NCHUNK - 1, L + k0 - H : L + k0 - H + W, :],
                        )
                        src = x_ap[b, 0:1, :]
                        bcast = bass.AP(
                            tensor=src.tensor,
                            offset=src.offset,
                            ap=[[0, 1], [0, W], [1, E]],
                        )
                        nc.sync.dma_start(out=xg[64 * bb : 64 * bb + 1, :, :], in_=bcast)
                    xg_cur[g] = xg

            xt_ps = xt_psp.tile([128, 2, 2, 128], FP32)   # [p, k%2, group, lane]

            def transpose_step(k):
                """PE-transpose one-hot for step k into xt_ps slot k%2, then
                cast-copy to an fp8 SBUF tile; returns the SBUF tile."""
                if k % W == 0:
                    prologue_load(k)
                for g in range(2):
                    nc.tensor.transpose(
                        xt_ps[:, k % 2, g, :], xg_cur[g][:, k % W, :], ident
                    )
                xt_sb = xtsb.tile([128, 2, 128], FP8, tag="xt_sb", name=f"xt_{k}")
                nc.scalar.copy(out=xt_sb, in_=xt_ps[:, k % 2, :, :])
                return xt_sb

            def e_matmul(k, xt_sb):
                """e_ps[:, j, :] = Bm8[:, :, j-tile].T (DoubleRow, slot1=0)
                @ [xt; xt] for all 256 lanes of step k."""
                e_ps = e_psp.tile([128, SJ, LANES], FP32, tag="e_ps", name=f"e_ps_{k}")
                rhs = _with_bcast2(xt_sb[:, :, :].rearrange("p g l -> p (g l)"))
                for j in range(SJ):
                    nc.tensor.matmul(
                        e_ps[:, j, :], lhsT=Bm8[:, :, ts(j, 128)], rhs=rhs,
                        start=True, stop=True, perf_mode=DR,
                    )
                return e_ps

            def e_copy(k, e_ps):
                """Stage emissions to SBUF bf16 (PSUM can't feed both inputs
                of the phi multiply)."""
                e_sb = esb.tile([128, SJ, LANES], BF16, tag="e_sb", name=f"e_sb_{k}")
                nc.scalar.copy(out=e_sb[:, 0:1, :], in_=e_ps[:, 0:1, :])
                nc.scalar.copy(out=e_sb[:, 1:4, :], in_=e_ps[:, 1:4, :])
                return e_sb

            # ---------------- pipeline priming ----------------
            xt_q = {0: transpose_step(0), 1: transpose_step(1)}
            e_ps_q = {0: e_matmul(0, xt_q[0])}
            e_sb_q = {0: e_copy(0, e_ps_q[0])}

            # ---------------- main lockstep scan ----------------
            phi = None
            for k in range(STEPS):
                e_sb = e_sb_q.pop(k)

                # advance the e pipeline first: its PE matmuls head the block
                # so the ACT copies (slowest chain) start as early as possible
                if k + 1 < STEPS:
                    e_ps_q[k + 1] = e_matmul(k + 1, xt_q.pop(k + 1))
                    e_sb_q[k + 1] = e_copy(k + 1, e_ps_q.pop(k + 1))

                phi_new = [
                    phip.tile([128, 2, LANES], FP8E5, tag=f"phi{jp}", name=f"phi{jp}_{k}")
                    for jp in range(2)
                ]

                if k == 0:
                    for jp in range(2):
                        nc.vector.tensor_copy(
                            out=phi_new[jp], in_=e_sb[:, 2 * jp : 2 * jp + 2, :]
                        )
                else:
                    # A-apply: 8 DoubleRow matmuls, j-outer (each j's PSUM
                    # closes after 2 instructions so its multiply starts early)
                    main0 = main_psp.tile([128, LANES], FP32, tag="main0", name=f"m0_{k}")
                    main1 = main_psp.tile([128, LANES], FP32, tag="main1", name=f"m1_{k}")
                    main23 = main_psp.tile([128, 2, LANES], FP32, tag="main23", name=f"m23_{k}")
                    mains = [main0, main1, main23[:, 0, :], main23[:, 1, :]]
                    for j in range(SJ):
                        for p in range(2):
                            nc.tensor.matmul(
                                mains[j], lhsT=A8[:, p, :, ts(j, 128)], rhs=phi[p],
                                start=(p == 0), stop=(p == 1), perf_mode=DR,
                            )
                        nc.vector.tensor_tensor(
                            out=phi_new[j // 2][:, j % 2, :],
                            in0=mains[j], in1=e_sb[:, j, :],
                            op=mybir.AluOpType.mult,
                        )

                if k + 2 < STEPS:
                    xt_q[k + 2] = transpose_step(k + 2)

                if k == H:
                    # exact re-seed of chunk-0 lanes (cols 0,64,128,192):
                    # phi[:, lane] = delta(state 0) * e_t0[:, lane] * CSEED
                    for jp in range(2):
                        for jo in range(2):
                            nc.vector.memset(
                                phi_new[jp][:, jo, :].rearrange(
                                    "p (b c) -> p b c", c=L
                                )[:, :, 0],
                                0.0,
                            )
                    nc.vector.tensor_scalar_mul(
                        out=phi_new[0][0:1, 0, :].rearrange(
                            "p (b c) -> p b c", c=L
                        )[:, :, 0],
                        in0=e_sb[0:1, 0, :].rearrange(
                            "p (b c) -> p b c", c=L
                        )[:, :, 0],
                        scalar1=float(CSEED),
                    )

                if k in (H - 1, STEPS - 1):
                    ck_ps = e_psp.tile([128, SJ, LANES], FP32, tag="e_ps",
                                       name=f"ck_{k}")
                    for p in range(2):
                        for s in range(2):
                            nc.tensor.matmul(
                                ck_ps[0:1, 0, :], lhsT=ones8,
                                rhs=phi_new[p][:, s, :],
                                start=(p == 0 and s == 0),
                                stop=(p == 1 and s == 1),
                            )
                    tgt = lnS_start if k == H - 1 else lnS_end
                    nc.scalar.activation(
                        out=tgt, in_=ck_ps[0:1, 0, :],
                        func=mybir.ActivationFunctionType.Ln,
                    )

                phi = phi_new

            # ---------------- combine ----------------
            u = prep.tile([1, LANES], FP32, tag="u")
            nc.vector.tensor_tensor(
                out=u, in0=lnS_end, in1=lnS_start, op=mybir.AluOpType.subtract
            )
            # chunk-0 lanes have no burn-in reference: u = lnS_end
            nc.vector.tensor_add(
                out=u.rearrange("p (b c) -> p b c", c=L)[:, :, 0],
                in0=u.rearrange("p (b c) -> p b c", c=L)[:, :, 0],
                in1=lnS_start.rearrange("p (b c) -> p b c", c=L)[:, :, 0],
            )
            ub = prep.tile([1, BPC], FP32, tag="ub")
            nc.vector.tensor_reduce(
                out=ub,
                in_=u.rearrange("p (b c) -> p b c", c=L),
                axis=mybir.AxisListType.X,
                op=mybir.AluOpType.add,
            )
            # undo the per-step prescale (every emission was multiplied by
            # exp(LNS)) and the chunk-0 reseed boost CSEED/ASCALE
            nc.vector.tensor_scalar_add(
                out=ub, in0=ub,
                scalar1=float(-T * LNS - np.log(CSEED / ASCALE)),
            )
            nc.sync.dma_start(out=out.ap(), in_=ub)

    nc.compile()
    return nc


_NC = None


def _get_nc():
    global _NC
    if _NC is None:
        _NC = build_program()
    return _NC


def kernel(inputs: np.ndarray, A_kernel: np.ndarray, B_kernel: np.ndarray) -> np.ndarray:
    nc = _get_nc()
    ak = np.ascontiguousarray(A_kernel, dtype=np.float32)
    bk = np.ascontiguousarray(B_kernel, dtype=np.float32)
    in_maps = [
        {
            "x": np.ascontiguousarray(inputs[BPC * c : BPC * (c + 1)], dtype=np.float32),
            "ak": ak,
            "bk": bk,
        }
        for c in range(NCORES)
    ]
    res = run_bass_kernel_spmd(nc, in_maps, core_ids=list(range(NCORES)))
    out = np.concatenate([res.results[c]["out"].reshape(BPC) for c in range(NCORES)])
    return out.reshape(B, 1).astype(np.float32)
